# revision 5
# baseline (speedup 1.0000x reference)
"""BERT encoder forward (nn_BERT_80805514707426) on 8 Trainium2 NeuronCores.

Strategy: data-parallel over batch (B=8 -> 1 sequence per core). Each core
runs the full 6-layer encoder on its sequence and writes its attention
probabilities + classifier output. Host shards inputs (embedding rows per
core's tokens), folds LayerNorm gains/biases into adjacent weights, and
gathers per-core results into the full outputs.

On-device layout: the residual stream is kept transposed, hT = h.T with
shape [D=768 (partitions, 6 tiles), S=512 (free)], so every GEMM uses the
weight matrix in its natural [d_in, d_out] layout as the stationary operand
and contracts over partitions. LayerNorm statistics (sums over D, i.e. over
partitions) are computed with ones-vector matmuls; per-token values are
broadcast across partitions with K=1 matmuls. Softmax skips the max
subtraction (scores are provably tiny for this model: |s| < ~2) and is
computed in both orientations: the [q, k] orientation produces the attention
output tensor with contiguous DMA writes, the [k, q] orientation feeds the
P @ V matmul without any transposes. All matmuls run in float32r (full PE
rate at N>=256, ~1e-4 relative rounding).
"""

import os
import sys

import numpy as np

for _p in ("/opt/trn_rl_repo", "/root/.axon_site/_ro/trn_rl_repo"):
    if os.path.isdir(_p) and _p not in sys.path:
        sys.path.append(_p)

import concourse.bacc as bacc
import concourse.tile as tile
from concourse import mybir
from concourse.bass_utils import run_bass_kernel_spmd

F32 = mybir.dt.float32
F32R = mybir.dt.float32r
AF = mybir.ActivationFunctionType
OP = mybir.AluOpType

B, S, D, H, L, FF, V, C = 8, 512, 768, 12, 6, 3072, 30522, 2
HD = D // H      # 64
KT = D // 128    # 6 d-model tiles
ST = S // 128    # 4 sequence tiles
FT = FF // 128   # 24 ffn tiles
EPS = 1e-5
N_CORES = 8

_CACHE = {}


# --------------------------------------------------------------------------
# device program
# --------------------------------------------------------------------------

def _emit_ln(nc, tc, l, tag, ht, hn_pool, small, ones, onesr):
    """hT (f32r, 6x[128,512]) -> LayerNorm'd hnT tiles (f32r). gamma/beta are
    folded into the consuming weights on the host, so this is the pure
    (x - mean) * rsqrt(var + eps) transform, with mean/var over partitions."""
    hn = []
    with (
        tc.tile_pool(name=f"ps_stat_{tag}", bufs=2, space="PSUM") as ps_stat,
        tc.tile_pool(name=f"ps_sq_{tag}", bufs=2, space="PSUM") as _,
        tc.tile_pool(name=f"ps_bc_{tag}", bufs=2, space="PSUM") as ps_bc,
        tc.tile_pool(name=f"sq_{tag}", bufs=3) as sq_pool,
    ):
        ssum = ps_stat.tile([1, S], F32, tag="stat")
        ssq = ps_stat.tile([1, S], F32, tag="stat")
        sqt = []
        for kt in range(KT):
            sq = sq_pool.tile([128, S], F32R, tag="sq")
            nc.scalar.square(sq[:], ht[kt][:])
            sqt.append(sq)
        for kt in range(KT):
            nc.tensor.matmul(ssum[:], ones[:], ht[kt][:],
                             start=(kt == 0), stop=(kt == KT - 1))
        for kt in range(KT):
            nc.tensor.matmul(ssq[:], ones[:], sqt[kt][:],
                             start=(kt == 0), stop=(kt == KT - 1))
        m = small.tile([1, S], F32, tag="lnrow", bufs=8, name="m")
        nc.vector.tensor_scalar_mul(m[:], ssum[:], 1.0 / D)
        msq = small.tile([1, S], F32, tag="lnrow", bufs=8, name="msq")
        nc.vector.tensor_mul(msq[:], m[:], m[:])
        var = small.tile([1, S], F32, tag="lnrow", bufs=8, name="var")
        nc.vector.scalar_tensor_tensor(var[:], ssq[:], 1.0 / D, msq[:],
                                       op0=OP.mult, op1=OP.subtract)
        vare = small.tile([1, S], F32, tag="lnrow", bufs=8, name="vare")
        nc.vector.tensor_scalar_add(vare[:], var[:], float(EPS))
        rec = small.tile([1, S], F32, tag="lnrow", bufs=8, name="rec")
        nc.vector.reciprocal(rec[:], vare[:])
        istd = small.tile([1, S], F32R, tag="lnrow", bufs=8, name="istd")
        nc.scalar.sqrt(istd[:], rec[:])
        mistd = small.tile([1, S], F32R, tag="lnrow", bufs=8, name="mistd")
        nc.vector.tensor_mul(mistd[:], m[:], istd[:])
        # broadcast istd / (m*istd) across partitions via K=1 matmuls
        istd_b = ps_bc.tile([128, S], F32, tag="bc")
        mistd_b = ps_bc.tile([128, S], F32, tag="bc")
        nc.tensor.matmul(istd_b[:], onesr[:], istd[:], start=True, stop=True)
        nc.tensor.matmul(mistd_b[:], onesr[:], mistd[:], start=True, stop=True)
        for kt in range(KT):
            tmp = sq_pool.tile([128, S], F32, tag="tmp")
            nc.vector.tensor_mul(tmp[:], ht[kt][:], istd_b[:])
            h = hn_pool.tile([128, S], F32R, tag="hn")
            nc.vector.tensor_sub(h[:], tmp[:], mistd_b[:])
            hn.append(h)
    return hn


def _emit_layer(nc, tc, l, ht, pools, consts, dram, mask_all_ones):
    (hn_pool, qk_pool, v_pool, et_pool, en_pool, at_pool, ot_pool, rzb_pool,
     zt_pool, wp_pool, w12_pool, bias_pool, small) = pools
    ones, onesr, kbc_sb, mask_b = consts
    WQ, WK, WV, WO, W1R, W2, BQ, BK, BO, BF1, BF2, ATT = dram

    # per-layer biases -> SBUF ([128, n] column layouts)
    bq_sb = bias_pool.tile([128, KT], F32, tag="bq")
    nc.sync.dma_start(bq_sb[:], BQ[l])
    bk_sb = bias_pool.tile([128, KT], F32, tag="bk")
    nc.sync.dma_start(bk_sb[:], BK[l])
    bo_sb = bias_pool.tile([128, KT], F32, tag="bo")
    nc.sync.dma_start(bo_sb[:], BO[l])
    bf1_sb = bias_pool.tile([128, FT], F32, tag="bf1")
    nc.sync.dma_start(bf1_sb[:], BF1[l])
    bf2_sb = bias_pool.tile([128, KT], F32, tag="bf2")
    nc.sync.dma_start(bf2_sb[:], BF2[l])

    # ---- LN1 ----
    hn = _emit_ln(nc, tc, l, f"l{l}a", ht, hn_pool, small, ones, onesr)

    # ---- QKV projections ----
    qT, kTt, vT = [], [], []
    with tc.tile_pool(name=f"ps_proj_{l}", bufs=3, space="PSUM") as ps_proj:
        for name, W, bsb, outs in (("q", WQ, bq_sb, qT), ("k", WK, bk_sb, kTt)):
            wsb = []
            for kt in range(KT):
                w = wp_pool.tile([128, D], F32R, tag="wp")
                nc.sync.dma_start(w[:], W[l, kt * 128:(kt + 1) * 128, :].bitcast(F32R))
                wsb.append(w)
            for j in range(KT):
                ps = ps_proj.tile([128, S], F32, tag="proj")
                for kt in range(KT):
                    nc.tensor.matmul(ps[:], wsb[kt][:, j * 128:(j + 1) * 128],
                                     hn[kt][:], start=(kt == 0), stop=(kt == KT - 1))
                o = qk_pool.tile([128, S], F32R, tag="qk")
                nc.vector.tensor_scalar_add(o[:], ps[:], bsb[:, j:j + 1])
                outs.append(o)
        # V in normal layout [tokens, d] (bias folded into bo on host)
        wsb = []
        for kt in range(KT):
            w = wp_pool.tile([128, D], F32R, tag="wp")
            nc.sync.dma_start(w[:], WV[l, kt * 128:(kt + 1) * 128, :].bitcast(F32R))
            wsb.append(w)
        for t in range(ST):
            v = v_pool.tile([128, D], F32R, tag="v")
            for nh in range(2):
                ps = ps_proj.tile([128, 384], F32, tag="projv")
                for kt in range(KT):
                    nc.tensor.matmul(ps[:], hn[kt][:, t * 128:(t + 1) * 128],
                                     wsb[kt][:, nh * 384:(nh + 1) * 384],
                                     start=(kt == 0), stop=(kt == KT - 1))
                nc.vector.tensor_copy(v[:, nh * 384:(nh + 1) * 384], ps[:])
            vT.append(v)

    # ---- attention heads ----
    oT = [ot_pool.tile([128, S], F32R, tag="ot", name=f"ot{l}_{j}")
          for j in range(KT)]
    with (
        tc.tile_pool(name=f"ps_sT_{l}", bufs=2, space="PSUM") as ps_sT,
        tc.tile_pool(name=f"ps_sn_{l}", bufs=2, space="PSUM") as ps_sn,
        tc.tile_pool(name=f"ps_z_{l}", bufs=2, space="PSUM") as ps_z,
        tc.tile_pool(name=f"ps_o_{l}", bufs=2, space="PSUM") as ps_o,
    ):
        for hh in range(H):
            jt, ro = hh // 2, (hh % 2) * 64  # tile and row offset of this head
            qh = qT[jt][ro:ro + HD, :]
            kh = kTt[jt][ro:ro + HD, :]
            # transposed scores + exp -> unnormalized probs P^T (f32r)
            eT = []
            for c in range(ST):
                ps = ps_sT.tile([128, S], F32, tag="sT")
                nc.tensor.matmul(ps[:], kh[:, c * 128:(c + 1) * 128], qh,
                                 start=True, stop=True)
                e = et_pool.tile([128, S], F32R, tag="et")
                if mask_all_ones:
                    nc.scalar.activation(e[:], ps[:], AF.Exp)
                else:
                    nc.scalar.activation(e[:], ps[:], AF.Exp,
                                         bias=kbc_sb[:, c:c + 1])
                eT.append(e)
            # Z per query (free layout) from ones-matmul over keys
            zf = ps_z.tile([1, S], F32, tag="zz")
            for c in range(ST):
                nc.tensor.matmul(zf[:], ones[:], eT[c][:],
                                 start=(c == 0), stop=(c == ST - 1))
            rzf = small.tile([1, S], F32R, tag="rzf", bufs=3)
            nc.vector.reciprocal(rzf[:], zf[:])
            rzb_ps = ps_z.tile([64, S], F32, tag="zz")
            nc.tensor.matmul(rzb_ps[:], onesr[0:1, 0:64], rzf[:],
                             start=True, stop=True)
            rzb = rzb_pool.tile([64, S], F32, tag="rzb")
            nc.vector.tensor_copy(rzb[:], rzb_ps[:])
            # normal-orientation scores -> attention output rows
            for c in range(ST):
                ps = ps_sn.tile([128, S], F32, tag="sn")
                nc.tensor.matmul(ps[:], qh[:, c * 128:(c + 1) * 128], kh,
                                 start=True, stop=True)
                en = en_pool.tile([128, S], F32, tag="en")
                zp = small.tile([128, 1], F32, tag="zp", bufs=6)
                if mask_all_ones:
                    nc.scalar.activation(en[:], ps[:], AF.Exp, accum_out=zp[:])
                else:
                    en0 = en_pool.tile([128, S], F32, tag="en")
                    nc.scalar.activation(en0[:], ps[:], AF.Exp)
                    nc.vector.scalar_tensor_tensor(en[:], en0[:], 1.0, mask_b[:],
                                                   op0=OP.mult, op1=OP.mult,
                                                   accum_out=zp[:])
                rzp = small.tile([128, 1], F32, tag="rzp", bufs=6)
                nc.vector.reciprocal(rzp[:], zp[:])
                at = at_pool.tile([128, S], F32, tag="at")
                nc.vector.tensor_scalar_mul(at[:], en[:], rzp[:])
                nc.sync.dma_start(ATT[l, hh, c * 128:(c + 1) * 128, :], at[:])
            # P^T @ V -> head output (transposed), normalized by Z
            po = ps_o.tile([64, S], F32, tag="po")
            for c in range(ST):
                nc.tensor.matmul(po[:], vT[c][:, hh * HD:(hh + 1) * HD], eT[c][:],
                                 start=(c == 0), stop=(c == ST - 1))
            nc.vector.tensor_mul(oT[jt][ro:ro + HD, :], po[:], rzb[:])

    # ---- output projection + residual ----
    ht2 = []
    with tc.tile_pool(name=f"ps_op_{l}", bufs=3, space="PSUM") as ps_op:
        wsb = []
        for kt in range(KT):
            w = wp_pool.tile([128, D], F32R, tag="wp")
            nc.sync.dma_start(w[:], WO[l, kt * 128:(kt + 1) * 128, :].bitcast(F32R))
            wsb.append(w)
        for j in range(KT):
            ps = ps_op.tile([128, S], F32, tag="op")
            for kt in range(KT):
                nc.tensor.matmul(ps[:], wsb[kt][:, j * 128:(j + 1) * 128],
                                 oT[kt][:], start=(kt == 0), stop=(kt == KT - 1))
            h = hn_pool.tile([128, S], F32R, tag="ht", bufs=13)
            nc.vector.scalar_tensor_tensor(h[:], ps[:], bo_sb[:, j:j + 1],
                                           ht[j][:], op0=OP.add, op1=OP.add)
            ht2.append(h)

    # ---- LN2 + FFN ----
    hn2 = _emit_ln(nc, tc, l, f"l{l}b", ht2, hn_pool, small, ones, onesr)
    ht3 = []
    with (
        tc.tile_pool(name=f"ps_ff_{l}", bufs=6, space="PSUM") as ps_ff,
        tc.tile_pool(name=f"ps_zg_{l}", bufs=2, space="PSUM") as ps_zg,
    ):
        ff_ps = [ps_ff.tile([128, S], F32, tag="ff", name=f"ff{l}_{j}")
                 for j in range(KT)]
        for i in range(FT):
            w1 = w12_pool.tile([128, D], F32R, tag="w1")
            nc.sync.dma_start(w1[:], W1R[l, i].bitcast(F32R))
            psz = ps_zg.tile([128, S], F32, tag="zg")
            for kt in range(KT):
                nc.tensor.matmul(psz[:], w1[:, kt * 128:(kt + 1) * 128],
                                 hn2[kt][:], start=(kt == 0), stop=(kt == KT - 1))
            zt = zt_pool.tile([128, S], F32R, tag="zt")
            nc.scalar.activation(zt[:], psz[:], AF.Gelu, bias=bf1_sb[:, i:i + 1])
            w2 = w12_pool.tile([128, D], F32R, tag="w2")
            nc.sync.dma_start(w2[:], W2[l, i * 128:(i + 1) * 128, :].bitcast(F32R))
            for j in range(KT):
                nc.tensor.matmul(ff_ps[j][:], w2[:, j * 128:(j + 1) * 128],
                                 zt[:], start=(i == 0), stop=(i == FT - 1))
        for j in range(KT):
            h = hn_pool.tile([128, S], F32R, tag="ht", bufs=13)
            nc.vector.scalar_tensor_tensor(h[:], ff_ps[j][:], bf2_sb[:, j:j + 1],
                                           ht2[j][:], op0=OP.add, op1=OP.add)
            ht3.append(h)
    return ht3


def _build(mask_all_ones: bool):
    nc = bacc.Bacc("TRN2", target_bir_lowering=False, debug=False,
                   num_devices=N_CORES)

    H0T = nc.dram_tensor("h0t", [D, S], F32, kind="ExternalInput")
    WQ = nc.dram_tensor("wq", [L, D, D], F32, kind="ExternalInput")
    WK = nc.dram_tensor("wk", [L, D, D], F32, kind="ExternalInput")
    WV = nc.dram_tensor("wv", [L, D, D], F32, kind="ExternalInput")
    WO = nc.dram_tensor("wo", [L, D, D], F32, kind="ExternalInput")
    W1R = nc.dram_tensor("w1r", [L, FT, 128, D], F32, kind="ExternalInput")
    W2 = nc.dram_tensor("w2", [L, FF, D], F32, kind="ExternalInput")
    BQ = nc.dram_tensor("bq", [L, 128, KT], F32, kind="ExternalInput")
    BK = nc.dram_tensor("bk", [L, 128, KT], F32, kind="ExternalInput")
    BO = nc.dram_tensor("bo", [L, 128, KT], F32, kind="ExternalInput")
    BF1 = nc.dram_tensor("bf1", [L, 128, FT], F32, kind="ExternalInput")
    BF2 = nc.dram_tensor("bf2", [L, 128, KT], F32, kind="ExternalInput")
    WC = nc.dram_tensor("wc", [D, C], F32, kind="ExternalInput")
    BC = nc.dram_tensor("bc", [1, C], F32, kind="ExternalInput")
    ONES = nc.dram_tensor("ones_col", [128, 1], F32, kind="ExternalInput")
    ONESR = nc.dram_tensor("ones_row", [1, 128], F32, kind="ExternalInput")
    KBC = nc.dram_tensor("kbcol", [128, ST], F32, kind="ExternalInput")
    M01 = nc.dram_tensor("mask01", [1, S], F32, kind="ExternalInput")
    ATT = nc.dram_tensor("attn", [L, H, S, S], F32, kind="ExternalOutput")
    OUT = nc.dram_tensor("out", [1, C], F32, kind="ExternalOutput")

    with tile.TileContext(nc) as tc, nc.allow_low_precision(
            reason="float32r matmul pipeline (tf32-like, validated vs ref)"):
        with (
            tc.tile_pool(name="consts", bufs=1) as cp,
            tc.tile_pool(name="bias", bufs=2) as bias_pool,
            tc.tile_pool(name="hn", bufs=7) as hn_pool,       # ht(13) + hn(7)
            tc.tile_pool(name="qk", bufs=12) as qk_pool,
            tc.tile_pool(name="v", bufs=5) as v_pool,
            tc.tile_pool(name="et", bufs=5) as et_pool,
            tc.tile_pool(name="en", bufs=3) as en_pool,
            tc.tile_pool(name="at", bufs=3) as at_pool,
            tc.tile_pool(name="ot", bufs=6) as ot_pool,
            tc.tile_pool(name="rzb", bufs=2) as rzb_pool,
            tc.tile_pool(name="zt", bufs=4) as zt_pool,
            tc.tile_pool(name="wp", bufs=7) as wp_pool,
            tc.tile_pool(name="w12", bufs=3) as w12_pool,
            tc.tile_pool(name="small", bufs=6) as small,
        ):
            ones = cp.tile([128, 1], F32R, tag="ones")
            nc.sync.dma_start(ones[:], ONES[:].bitcast(F32R))
            onesr = cp.tile([1, 128], F32R, tag="onesr")
            nc.sync.dma_start(onesr[:], ONESR[:].bitcast(F32R))
            kbc_sb = None
            mask_b = None
            if not mask_all_ones:
                kbc_sb = cp.tile([128, ST], F32, tag="kbc")
                nc.sync.dma_start(kbc_sb[:], KBC[:])
                m01r = cp.tile([1, S], F32R, tag="m01")
                nc.sync.dma_start(m01r[:], M01[:].bitcast(F32R))
                with tc.tile_pool(name="ps_m", bufs=1, space="PSUM") as ps_m:
                    mb_ps = ps_m.tile([128, S], F32, tag="mb")
                    nc.tensor.matmul(mb_ps[:], onesr[:], m01r[:],
                                     start=True, stop=True)
                    mask_b = cp.tile([128, S], F32, tag="maskb")
                    nc.vector.tensor_copy(mask_b[:], mb_ps[:])

            ht = []
            for kt in range(KT):
                h = hn_pool.tile([128, S], F32R, tag="ht", bufs=13)
                nc.sync.dma_start(h[:], H0T[kt * 128:(kt + 1) * 128, :].bitcast(F32R))
                ht.append(h)

            pools = (hn_pool, qk_pool, v_pool, et_pool, en_pool, at_pool,
                     ot_pool, rzb_pool, zt_pool, wp_pool, w12_pool, bias_pool,
                     small)
            consts = (ones, onesr, kbc_sb, mask_b)
            dram = (WQ, WK, WV, WO, W1R, W2, BQ, BK, BO, BF1, BF2, ATT)
            for l in range(L):
                ht = _emit_layer(nc, tc, l, ht, pools, consts, dram,
                                 mask_all_ones)

            # ---- classifier head on the CLS token (token 0) ----
            with (
                tc.tile_pool(name="ps_cls", bufs=2, space="PSUM") as ps_cls,
                tc.tile_pool(name="cls", bufs=1) as clsp,
            ):
                # fp32r matmuls need even free sizes: run the CLS stats over
                # token columns 0:2 and use column 0.
                csum = ps_cls.tile([1, 2], F32, tag="cstat")
                cssq = ps_cls.tile([1, 2], F32, tag="cstat")
                sqc = []
                for kt in range(KT):
                    sq = clsp.tile([128, 2], F32R, tag=f"csq{kt}")
                    nc.scalar.square(sq[:], ht[kt][:, 0:2])
                    sqc.append(sq)
                for kt in range(KT):
                    nc.tensor.matmul(csum[:], ones[:], ht[kt][:, 0:2],
                                     start=(kt == 0), stop=(kt == KT - 1))
                for kt in range(KT):
                    nc.tensor.matmul(cssq[:], ones[:], sqc[kt][:],
                                     start=(kt == 0), stop=(kt == KT - 1))
                m = clsp.tile([1, 2], F32, tag="cm")
                nc.vector.tensor_scalar_mul(m[:], csum[:], 1.0 / D)
                msq = clsp.tile([1, 2], F32, tag="cmsq")
                nc.vector.tensor_mul(msq[:], m[:], m[:])
                var = clsp.tile([1, 2], F32, tag="cvar")
                nc.vector.scalar_tensor_tensor(var[:], cssq[:], 1.0 / D, msq[:],
                                               op0=OP.mult, op1=OP.subtract)
                vare = clsp.tile([1, 2], F32, tag="cvare")
                nc.vector.tensor_scalar_add(vare[:], var[:], float(EPS))
                rec = clsp.tile([1, 2], F32, tag="crec")
                nc.vector.reciprocal(rec[:], vare[:])
                istd = clsp.tile([1, 2], F32R, tag="cistd")
                nc.scalar.sqrt(istd[:], rec[:])
                mistd = clsp.tile([1, 2], F32R, tag="cmistd")
                nc.vector.tensor_mul(mistd[:], m[:], istd[:])
                istd_b = ps_cls.tile([128, 2], F32, tag="cbc")
                mistd_b = ps_cls.tile([128, 2], F32, tag="cbc")
                nc.tensor.matmul(istd_b[:], onesr[:], istd[:], start=True, stop=True)
                nc.tensor.matmul(mistd_b[:], onesr[:], mistd[:], start=True, stop=True)
                istd_s = clsp.tile([128, 1], F32, tag="cistds")
                nc.vector.tensor_copy(istd_s[:], istd_b[:, 0:1])
                mistd_s = clsp.tile([128, 1], F32, tag="cmistds")
                nc.vector.tensor_copy(mistd_s[:], mistd_b[:, 0:1])
                pout = ps_cls.tile([1, C], F32, tag="cout", bufs=1)
                for kt in range(KT):
                    cls0 = clsp.tile([128, 1], F32R, tag=f"cls0{kt}")
                    nc.vector.tensor_scalar(cls0[:], ht[kt][:, 0:1], istd_s[:],
                                            mistd_s[:], op0=OP.mult,
                                            op1=OP.subtract)
                    wc = clsp.tile([128, C], F32R, tag=f"wc{kt}")
                    nc.sync.dma_start(wc[:], WC[kt * 128:(kt + 1) * 128, :].bitcast(F32R))
                    nc.tensor.matmul(pout[:], cls0[:], wc[:],
                                     start=(kt == 0), stop=(kt == KT - 1))
                bc_sb = clsp.tile([1, C], F32, tag="cbias")
                nc.sync.dma_start(bc_sb[:], BC[:])
                orow = clsp.tile([1, C], F32, tag="orow")
                nc.vector.tensor_add(orow[:], pout[:], bc_sb[:])
                nc.sync.dma_start(OUT[:], orow[:])

    nc.compile()
    return nc


# --------------------------------------------------------------------------
# host side
# --------------------------------------------------------------------------

def _pos_enc():
    pos = np.arange(B, dtype=np.float32)[:, None]
    div = np.exp(np.arange(0, D, 2, dtype=np.float32) * (-np.log(10000.0) / D))
    pe = np.zeros((B, D), np.float32)
    pe[:, 0::2] = np.sin(pos * div)
    pe[:, 1::2] = np.cos(pos * div)
    return pe


def _prep_shared(inputs):
    """Fold LN gains/biases into adjacent weights; restage for the device."""
    f32 = lambda a: np.ascontiguousarray(np.asarray(a), dtype=np.float32)
    Wq, bq = f32(inputs["Wq"]), f32(inputs["bq"])
    Wk, bk = f32(inputs["Wk"]), f32(inputs["bk"])
    Wv, bv = f32(inputs["Wv"]), f32(inputs["bv"])
    Wo, bo = f32(inputs["Wo"]), f32(inputs["bo"])
    W1, bf1 = f32(inputs["W1"]), f32(inputs["bf1"])
    W2, bf2 = f32(inputs["W2"]), f32(inputs["bf2"])
    g1, b1 = f32(inputs["ln1_g"]), f32(inputs["ln1_b"])
    g2, b2 = f32(inputs["ln2_g"]), f32(inputs["ln2_b"])
    scale = np.float32(1.0 / np.sqrt(HD))

    wq = np.empty((L, D, D), np.float32)
    wk = np.empty((L, D, D), np.float32)
    wv = np.empty((L, D, D), np.float32)
    w1r = np.empty((L, FT, 128, D), np.float32)
    bqc = np.empty((L, 128, KT), np.float32)
    bkc = np.empty((L, 128, KT), np.float32)
    boc = np.empty((L, 128, KT), np.float32)
    bf1c = np.empty((L, 128, FT), np.float32)
    bf2c = np.empty((L, 128, KT), np.float32)
    col = lambda v, n: v.reshape(n, 128).T

    for l in range(L):
        wq[l] = (g1[l][:, None] * Wq[l]) * scale
        wk[l] = g1[l][:, None] * Wk[l]
        wv[l] = g1[l][:, None] * Wv[l]
        bql = (bq[l] + b1[l] @ Wq[l]) * scale
        bkl = bk[l] + b1[l] @ Wk[l]
        bvl = bv[l] + b1[l] @ Wv[l]
        bol = bo[l] + bvl @ Wo[l]
        w1l = g2[l][:, None] * W1[l]
        bf1l = bf1[l] + b2[l] @ W1[l]
        for i in range(FT):
            for kt in range(KT):
                w1r[l, i, :, kt * 128:(kt + 1) * 128] = \
                    w1l[kt * 128:(kt + 1) * 128, i * 128:(i + 1) * 128]
        bqc[l], bkc[l], boc[l] = col(bql, KT), col(bkl, KT), col(bol, KT)
        bf1c[l], bf2c[l] = col(bf1l, FT), col(bf2[l], KT)

    lncg, lncb = f32(inputs["lnc_g"]), f32(inputs["lnc_b"])
    Wc, bc = f32(inputs["Wc"]), f32(inputs["bc"])
    wc = lncg[:, None] * Wc
    bcf = (bc + lncb @ Wc).reshape(1, C)

    return {
        "wq": wq, "wk": wk, "wv": wv, "wo": Wo,
        "w1r": w1r, "w2": W2,
        "bq": bqc, "bk": bkc, "bo": boc, "bf1": bf1c, "bf2": bf2c,
        "wc": np.ascontiguousarray(wc), "bc": np.ascontiguousarray(bcf),
        "ones_col": np.ones((128, 1), np.float32),
        "ones_row": np.ones((1, 128), np.float32),
    }


def kernel(**inputs):
    x = np.asarray(inputs["x"])
    mask = np.asarray(inputs["mask"])
    emb = np.ascontiguousarray(np.asarray(inputs["emb"]), dtype=np.float32)
    mask_all_ones = bool(np.all(mask != 0))

    shared = _prep_shared(inputs)
    pe = _pos_enc()

    key = ("prog", mask_all_ones)
    if key not in _CACHE:
        _CACHE[key] = _build(mask_all_ones)
    nc = _CACHE[key]

    in_maps = []
    for b in range(B):
        m = dict(shared)
        h0 = emb[x[b]] + pe[b][None, :]                       # [S, D] fp32
        m["h0t"] = np.ascontiguousarray(h0.T)                 # [D, S]
        kb = np.where(mask[b] != 0, 0.0, -1e9).astype(np.float32)
        m["kbcol"] = np.ascontiguousarray(kb.reshape(ST, 128).T)
        m["mask01"] = (mask[b] != 0).astype(np.float32).reshape(1, S)
        in_maps.append(m)

    trace = bool(os.environ.get("BERT_TRACE"))
    if trace:
        _install_trace_shim()
    res = run_bass_kernel_spmd(nc, in_maps, core_ids=list(range(N_CORES)),
                               trace=trace)
    if trace:
        print(f"HW exec time: {res.exec_time_ns} ns")

    out = np.concatenate([res.results[b]["out"] for b in range(B)], axis=0)
    attns = np.stack([res.results[b]["attn"] for b in range(B)], axis=1)
    return out, attns


def _install_trace_shim():
    """The agent image's antenv lacks axon_hooks; register the NTFF profile
    hook directly from the boot helpers so trace=True works."""
    import types
    if "antenv.axon_hooks" in sys.modules:
        return
    try:
        from trn_agent_boot.trn_boot import _ntff_profile_via_ctypes
        hook = _ntff_profile_via_ctypes("/opt/axon/libaxon_pjrt.so")
    except Exception:
        hook = None
    mod = types.ModuleType("antenv.axon_hooks")
    mod.get_axon_ntff_profile_hook = lambda: hook
    sys.modules["antenv.axon_hooks"] = mod


# revision 7
# speedup vs baseline: 1.0405x; 1.0405x over previous
"""BERT encoder forward (nn_BERT_80805514707426) on 8 Trainium2 NeuronCores.

Strategy: data-parallel over batch (B=8 -> 1 sequence per core). Each core
runs the full 6-layer encoder on its sequence and writes its attention
probabilities + classifier output. Host shards inputs (embedding rows per
core's tokens), folds LayerNorm gains/biases into adjacent weights, and
gathers per-core results into the full outputs.

On-device layout: the residual stream is kept transposed, hT = h.T with
shape [D=768 (partitions, 6 tiles), S=512 (free)], so every GEMM uses the
weight matrix in its natural [d_in, d_out] layout as the stationary operand
and contracts over partitions. LayerNorm statistics (sums over D, i.e. over
partitions) are computed with ones-vector matmuls; per-token values are
broadcast across partitions with K=1 matmuls. Softmax skips the max
subtraction (scores are provably tiny for this model: |s| < ~2) and is
computed in both orientations: the [q, k] orientation produces the attention
output tensor with contiguous DMA writes, the [k, q] orientation feeds the
P @ V matmul without any transposes. All matmuls run in float32r (full PE
rate at N>=256, ~1e-4 relative rounding).
"""

import os
import sys

import numpy as np

for _p in ("/opt/trn_rl_repo", "/root/.axon_site/_ro/trn_rl_repo"):
    if os.path.isdir(_p) and _p not in sys.path:
        sys.path.append(_p)

import concourse.bacc as bacc
import concourse.tile as tile
from concourse import mybir
from concourse.bass_utils import run_bass_kernel_spmd

F32 = mybir.dt.float32
F32R = mybir.dt.float32r
AF = mybir.ActivationFunctionType
OP = mybir.AluOpType

B, S, D, H, L, FF, V, C = 8, 512, 768, 12, 6, 3072, 30522, 2
HD = D // H      # 64
KT = D // 128    # 6 d-model tiles
ST = S // 128    # 4 sequence tiles
FT = FF // 128   # 24 ffn tiles
EPS = 1e-5
N_CORES = 8

_CACHE = {}


# --------------------------------------------------------------------------
# device program
# --------------------------------------------------------------------------

def _emit_ln(nc, tc, l, tag, ht, hn_pool, small, ones, onesr):
    """hT (f32r, 6x[128,512]) -> LayerNorm'd hnT tiles (f32r). gamma/beta are
    folded into the consuming weights on the host, so this is the pure
    (x - mean) * rsqrt(var + eps) transform, with mean/var over partitions."""
    hn = []
    with (
        tc.tile_pool(name=f"ps_stat_{tag}", bufs=2, space="PSUM") as ps_stat,
        tc.tile_pool(name=f"ps_sq_{tag}", bufs=2, space="PSUM") as _,
        tc.tile_pool(name=f"ps_bc_{tag}", bufs=2, space="PSUM") as ps_bc,
        tc.tile_pool(name=f"sq_{tag}", bufs=3) as sq_pool,
    ):
        ssum = ps_stat.tile([1, S], F32, tag="stat")
        ssq = ps_stat.tile([1, S], F32, tag="stat")
        sqt = []
        for kt in range(KT):
            sq = sq_pool.tile([128, S], F32R, tag="sq", bufs=2)
            nc.gpsimd.tensor_mul(sq[:], ht[kt][:], ht[kt][:])
            sqt.append(sq)
        for kt in range(KT):
            nc.tensor.matmul(ssum[:], ones[:], ht[kt][:],
                             start=(kt == 0), stop=(kt == KT - 1))
        for kt in range(KT):
            nc.tensor.matmul(ssq[:], ones[:], sqt[kt][:],
                             start=(kt == 0), stop=(kt == KT - 1))
        m = small.tile([1, S], F32, tag="lnrow", bufs=8, name="m")
        nc.vector.tensor_scalar_mul(m[:], ssum[:], 1.0 / D)
        msq = small.tile([1, S], F32, tag="lnrow", bufs=8, name="msq")
        nc.vector.tensor_mul(msq[:], m[:], m[:])
        var = small.tile([1, S], F32, tag="lnrow", bufs=8, name="var")
        nc.vector.scalar_tensor_tensor(var[:], ssq[:], 1.0 / D, msq[:],
                                       op0=OP.mult, op1=OP.subtract)
        vare = small.tile([1, S], F32, tag="lnrow", bufs=8, name="vare")
        nc.vector.tensor_scalar_add(vare[:], var[:], float(EPS))
        rec = small.tile([1, S], F32, tag="lnrow", bufs=8, name="rec")
        nc.vector.reciprocal(rec[:], vare[:])
        istd = small.tile([1, S], F32R, tag="lnrow", bufs=8, name="istd")
        nc.scalar.sqrt(istd[:], rec[:])
        mistd = small.tile([1, S], F32R, tag="lnrow", bufs=8, name="mistd")
        nc.vector.tensor_mul(mistd[:], m[:], istd[:])
        # broadcast istd / (m*istd) across partitions via K=1 matmuls
        istd_b = ps_bc.tile([128, S], F32, tag="bc")
        mistd_b = ps_bc.tile([128, S], F32, tag="bc")
        nc.tensor.matmul(istd_b[:], onesr[:], istd[:], start=True, stop=True)
        nc.tensor.matmul(mistd_b[:], onesr[:], mistd[:], start=True, stop=True)
        for kt in range(KT):
            tmp = sq_pool.tile([128, S], F32, tag="tmp", bufs=2)
            nc.vector.tensor_mul(tmp[:], ht[kt][:], istd_b[:])
            h = hn_pool.tile([128, S], F32R, tag="hn")
            nc.vector.tensor_sub(h[:], tmp[:], mistd_b[:])
            hn.append(h)
    return hn


def _emit_layer(nc, tc, l, ht, pools, consts, dram, mask_all_ones):
    (hn_pool, qk_pool, v_pool, et_pool, en_pool, at_pool, ot_pool, rzb_pool,
     zt_pool, wp_pool, w12_pool, bias_pool, small) = pools
    ones, onesr, kbc_sb, mask_b = consts
    WQ, WK, WV, WO, W1R, W2, BQ, BK, BO, BF1, BF2, ATT = dram

    # per-layer biases -> SBUF ([128, n] column layouts)
    bq_sb = bias_pool.tile([128, KT], F32, tag="bq")
    nc.sync.dma_start(bq_sb[:], BQ[l])
    bk_sb = bias_pool.tile([128, KT], F32, tag="bk")
    nc.sync.dma_start(bk_sb[:], BK[l])
    bo_sb = bias_pool.tile([128, KT], F32, tag="bo")
    nc.sync.dma_start(bo_sb[:], BO[l])
    bf1_sb = bias_pool.tile([128, FT], F32, tag="bf1")
    nc.sync.dma_start(bf1_sb[:], BF1[l])
    bf2_sb = bias_pool.tile([128, KT], F32, tag="bf2")
    nc.sync.dma_start(bf2_sb[:], BF2[l])

    # ---- LN1 ----
    hn = _emit_ln(nc, tc, l, f"l{l}a", ht, hn_pool, small, ones, onesr)

    # ---- QKV projections ----
    qT, kTt, vT = [], [], []
    with tc.tile_pool(name=f"ps_proj_{l}", bufs=3, space="PSUM") as ps_proj:
        for name, W, bsb, outs in (("q", WQ, bq_sb, qT), ("k", WK, bk_sb, kTt)):
            wsb = []
            for kt in range(KT):
                w = wp_pool.tile([128, D], F32R, tag="wp")
                nc.sync.dma_start(w[:], W[l, kt * 128:(kt + 1) * 128, :].bitcast(F32R))
                wsb.append(w)
            for j in range(KT):
                ps = ps_proj.tile([128, S], F32, tag="proj")
                for kt in range(KT):
                    nc.tensor.matmul(ps[:], wsb[kt][:, j * 128:(j + 1) * 128],
                                     hn[kt][:], start=(kt == 0), stop=(kt == KT - 1))
                o = qk_pool.tile([128, S], F32R, tag="qk")
                nc.vector.tensor_scalar_add(o[:], ps[:], bsb[:, j:j + 1])
                outs.append(o)
        # V in normal layout [tokens, d] (bias folded into bo on host)
        wsb = []
        for kt in range(KT):
            w = wp_pool.tile([128, D], F32R, tag="wp")
            nc.sync.dma_start(w[:], WV[l, kt * 128:(kt + 1) * 128, :].bitcast(F32R))
            wsb.append(w)
        for t in range(ST):
            v = v_pool.tile([128, D], F32R, tag="v")
            for nh in range(2):
                ps = ps_proj.tile([128, 384], F32, tag="projv")
                for kt in range(KT):
                    nc.tensor.matmul(ps[:], hn[kt][:, t * 128:(t + 1) * 128],
                                     wsb[kt][:, nh * 384:(nh + 1) * 384],
                                     start=(kt == 0), stop=(kt == KT - 1))
                nc.vector.tensor_copy(v[:, nh * 384:(nh + 1) * 384], ps[:])
            vT.append(v)

    # ---- attention heads ----
    oT = [ot_pool.tile([128, S], F32R, tag="ot", name=f"ot{l}_{j}")
          for j in range(KT)]
    with (
        tc.tile_pool(name=f"ps_sT_{l}", bufs=2, space="PSUM") as ps_sT,
        tc.tile_pool(name=f"ps_sn_{l}", bufs=2, space="PSUM") as ps_sn,
        tc.tile_pool(name=f"ps_z_{l}", bufs=1, space="PSUM") as ps_z,
        tc.tile_pool(name=f"ps_o_{l}", bufs=1, space="PSUM") as ps_o,
    ):
        for hh in range(H):
            jt, ro = hh // 2, (hh % 2) * 64  # tile and row offset of this head
            qh = qT[jt][ro:ro + HD, :]
            kh = kTt[jt][ro:ro + HD, :]
            # transposed scores + exp -> unnormalized probs P^T (f32r).
            # Two key-chunks per PSUM tile so each exp covers [128, 1024].
            eT = []
            for cp in range(ST // 2):
                ps = ps_sT.tile([128, 2 * S], F32, tag="sT")
                for h2 in range(2):
                    c = 2 * cp + h2
                    nc.tensor.matmul(ps[:, h2 * S:(h2 + 1) * S],
                                     kh[:, c * 128:(c + 1) * 128], qh,
                                     start=True, stop=True)
                e = et_pool.tile([128, 2 * S], F32R, tag="et")
                if mask_all_ones:
                    nc.scalar.activation(e[:], ps[:], AF.Exp)
                else:
                    nc.scalar.activation(e[:, 0:S], ps[:, 0:S], AF.Exp,
                                         bias=kbc_sb[:, 2 * cp:2 * cp + 1])
                    nc.scalar.activation(e[:, S:2 * S], ps[:, S:2 * S], AF.Exp,
                                         bias=kbc_sb[:, 2 * cp + 1:2 * cp + 2])
                eT.append(e)
            ech = [eT[c // 2][:, (c % 2) * S:((c % 2) + 1) * S] for c in range(ST)]
            # Z per query (free layout) from ones-matmul over keys
            zf = ps_z.tile([1, S], F32, tag="zz")
            for c in range(ST):
                nc.tensor.matmul(zf[:], ones[:], ech[c],
                                 start=(c == 0), stop=(c == ST - 1))
            rzf = small.tile([1, S], F32R, tag="rzf", bufs=3)
            nc.vector.reciprocal(rzf[:], zf[:])
            rzb_ps = ps_z.tile([64, S], F32, tag="zz")
            nc.tensor.matmul(rzb_ps[:], onesr[0:1, 0:64], rzf[:],
                             start=True, stop=True)
            rzb = rzb_pool.tile([64, S], F32, tag="rzb")
            nc.vector.tensor_copy(rzb[:], rzb_ps[:])
            # normal-orientation scores -> attention output rows
            zp4 = small.tile([128, ST], F32, tag="zp", bufs=4)
            en4 = []
            for c in range(ST):
                ps = ps_sn.tile([128, S], F32, tag="sn")
                nc.tensor.matmul(ps[:], qh[:, c * 128:(c + 1) * 128], kh,
                                 start=True, stop=True)
                en = en_pool.tile([128, S], F32, tag="en")
                if mask_all_ones:
                    nc.scalar.activation(en[:], ps[:], AF.Exp,
                                         accum_out=zp4[:, c:c + 1])
                else:
                    en0 = en_pool.tile([128, S], F32, tag="en")
                    nc.scalar.activation(en0[:], ps[:], AF.Exp)
                    nc.vector.scalar_tensor_tensor(en[:], en0[:], 1.0, mask_b[:],
                                                   op0=OP.mult, op1=OP.mult,
                                                   accum_out=zp4[:, c:c + 1])
                en4.append(en)
            rzp4 = small.tile([128, ST], F32, tag="rzp", bufs=4)
            nc.vector.reciprocal(rzp4[:], zp4[:])
            at4 = at_pool.tile([128, ST, S], F32, tag="at")
            for c in range(ST):
                nc.vector.tensor_scalar_mul(at4[:, c, :], en4[c][:],
                                            rzp4[:, c:c + 1])
            # one DMA for the whole head: SBUF [128,(4,512)] -> DRAM [512,512]
            nc.sync.dma_start(
                ATT[l, hh].rearrange("(c p) k -> p c k", p=128), at4[:])
            # P^T @ V -> head output (transposed), normalized by Z
            po = ps_o.tile([64, S], F32, tag="po")
            for c in range(ST):
                nc.tensor.matmul(po[:], vT[c][:, hh * HD:(hh + 1) * HD], ech[c],
                                 start=(c == 0), stop=(c == ST - 1))
            nc.vector.tensor_mul(oT[jt][ro:ro + HD, :], po[:], rzb[:])

    # ---- output projection + residual ----
    ht2 = []
    with tc.tile_pool(name=f"ps_op_{l}", bufs=3, space="PSUM") as ps_op:
        wsb = []
        for kt in range(KT):
            w = wp_pool.tile([128, D], F32R, tag="wp")
            nc.sync.dma_start(w[:], WO[l, kt * 128:(kt + 1) * 128, :].bitcast(F32R))
            wsb.append(w)
        for j in range(KT):
            ps = ps_op.tile([128, S], F32, tag="op")
            for kt in range(KT):
                nc.tensor.matmul(ps[:], wsb[kt][:, j * 128:(j + 1) * 128],
                                 oT[kt][:], start=(kt == 0), stop=(kt == KT - 1))
            h = hn_pool.tile([128, S], F32R, tag="ht", bufs=13)
            nc.vector.scalar_tensor_tensor(h[:], ps[:], bo_sb[:, j:j + 1],
                                           ht[j][:], op0=OP.add, op1=OP.add)
            ht2.append(h)

    # ---- LN2 + FFN ----
    hn2 = _emit_ln(nc, tc, l, f"l{l}b", ht2, hn_pool, small, ones, onesr)
    ht3 = []
    with (
        tc.tile_pool(name=f"ps_ff_{l}", bufs=6, space="PSUM") as ps_ff,
        tc.tile_pool(name=f"ps_zg_{l}", bufs=2, space="PSUM") as ps_zg,
    ):
        ff_ps = [ps_ff.tile([128, S], F32, tag="ff", name=f"ff{l}_{j}")
                 for j in range(KT)]
        for i in range(FT):
            w1 = w12_pool.tile([128, D], F32R, tag="w1")
            nc.sync.dma_start(w1[:], W1R[l, i].bitcast(F32R))
            psz = ps_zg.tile([128, S], F32, tag="zg")
            for kt in range(KT):
                nc.tensor.matmul(psz[:], w1[:, kt * 128:(kt + 1) * 128],
                                 hn2[kt][:], start=(kt == 0), stop=(kt == KT - 1))
            zt = zt_pool.tile([128, S], F32R, tag="zt")
            nc.scalar.activation(zt[:], psz[:], AF.Gelu, bias=bf1_sb[:, i:i + 1])
            w2 = w12_pool.tile([128, D], F32R, tag="w2")
            nc.sync.dma_start(w2[:], W2[l, i * 128:(i + 1) * 128, :].bitcast(F32R))
            for j in range(KT):
                nc.tensor.matmul(ff_ps[j][:], w2[:, j * 128:(j + 1) * 128],
                                 zt[:], start=(i == 0), stop=(i == FT - 1))
        for j in range(KT):
            h = hn_pool.tile([128, S], F32R, tag="ht", bufs=13)
            nc.vector.scalar_tensor_tensor(h[:], ff_ps[j][:], bf2_sb[:, j:j + 1],
                                           ht2[j][:], op0=OP.add, op1=OP.add)
            ht3.append(h)
    return ht3


def _build(mask_all_ones: bool):
    nc = bacc.Bacc("TRN2", target_bir_lowering=False, debug=False,
                   num_devices=N_CORES)

    H0T = nc.dram_tensor("h0t", [D, S], F32, kind="ExternalInput")
    WQ = nc.dram_tensor("wq", [L, D, D], F32, kind="ExternalInput")
    WK = nc.dram_tensor("wk", [L, D, D], F32, kind="ExternalInput")
    WV = nc.dram_tensor("wv", [L, D, D], F32, kind="ExternalInput")
    WO = nc.dram_tensor("wo", [L, D, D], F32, kind="ExternalInput")
    W1R = nc.dram_tensor("w1r", [L, FT, 128, D], F32, kind="ExternalInput")
    W2 = nc.dram_tensor("w2", [L, FF, D], F32, kind="ExternalInput")
    BQ = nc.dram_tensor("bq", [L, 128, KT], F32, kind="ExternalInput")
    BK = nc.dram_tensor("bk", [L, 128, KT], F32, kind="ExternalInput")
    BO = nc.dram_tensor("bo", [L, 128, KT], F32, kind="ExternalInput")
    BF1 = nc.dram_tensor("bf1", [L, 128, FT], F32, kind="ExternalInput")
    BF2 = nc.dram_tensor("bf2", [L, 128, KT], F32, kind="ExternalInput")
    WC = nc.dram_tensor("wc", [D, C], F32, kind="ExternalInput")
    BC = nc.dram_tensor("bc", [1, C], F32, kind="ExternalInput")
    ONES = nc.dram_tensor("ones_col", [128, 1], F32, kind="ExternalInput")
    ONESR = nc.dram_tensor("ones_row", [1, 128], F32, kind="ExternalInput")
    KBC = nc.dram_tensor("kbcol", [128, ST], F32, kind="ExternalInput")
    M01 = nc.dram_tensor("mask01", [1, S], F32, kind="ExternalInput")
    ATT = nc.dram_tensor("attn", [L, H, S, S], F32, kind="ExternalOutput")
    OUT = nc.dram_tensor("out", [1, C], F32, kind="ExternalOutput")

    with tile.TileContext(nc) as tc, nc.allow_low_precision(
            reason="float32r matmul pipeline (tf32-like, validated vs ref)"):
        with (
            tc.tile_pool(name="consts", bufs=1) as cp,
            tc.tile_pool(name="bias", bufs=2) as bias_pool,
            tc.tile_pool(name="hn", bufs=7) as hn_pool,       # ht(13) + hn(7)
            tc.tile_pool(name="qk", bufs=12) as qk_pool,
            tc.tile_pool(name="v", bufs=4) as v_pool,
            tc.tile_pool(name="et", bufs=3) as et_pool,
            tc.tile_pool(name="en", bufs=5) as en_pool,
            tc.tile_pool(name="at", bufs=2) as at_pool,
            tc.tile_pool(name="ot", bufs=6) as ot_pool,
            tc.tile_pool(name="rzb", bufs=2) as rzb_pool,
            tc.tile_pool(name="zt", bufs=3) as zt_pool,
            tc.tile_pool(name="wp", bufs=7) as wp_pool,
            tc.tile_pool(name="w12", bufs=3) as w12_pool,
            tc.tile_pool(name="small", bufs=6) as small,
        ):
            ones = cp.tile([128, 1], F32R, tag="ones")
            nc.sync.dma_start(ones[:], ONES[:].bitcast(F32R))
            onesr = cp.tile([1, 128], F32R, tag="onesr")
            nc.sync.dma_start(onesr[:], ONESR[:].bitcast(F32R))
            kbc_sb = None
            mask_b = None
            if not mask_all_ones:
                kbc_sb = cp.tile([128, ST], F32, tag="kbc")
                nc.sync.dma_start(kbc_sb[:], KBC[:])
                m01r = cp.tile([1, S], F32R, tag="m01")
                nc.sync.dma_start(m01r[:], M01[:].bitcast(F32R))
                with tc.tile_pool(name="ps_m", bufs=1, space="PSUM") as ps_m:
                    mb_ps = ps_m.tile([128, S], F32, tag="mb")
                    nc.tensor.matmul(mb_ps[:], onesr[:], m01r[:],
                                     start=True, stop=True)
                    mask_b = cp.tile([128, S], F32, tag="maskb")
                    nc.vector.tensor_copy(mask_b[:], mb_ps[:])

            ht = []
            for kt in range(KT):
                h = hn_pool.tile([128, S], F32R, tag="ht", bufs=13)
                nc.sync.dma_start(h[:], H0T[kt * 128:(kt + 1) * 128, :].bitcast(F32R))
                ht.append(h)

            pools = (hn_pool, qk_pool, v_pool, et_pool, en_pool, at_pool,
                     ot_pool, rzb_pool, zt_pool, wp_pool, w12_pool, bias_pool,
                     small)
            consts = (ones, onesr, kbc_sb, mask_b)
            dram = (WQ, WK, WV, WO, W1R, W2, BQ, BK, BO, BF1, BF2, ATT)
            for l in range(L):
                ht = _emit_layer(nc, tc, l, ht, pools, consts, dram,
                                 mask_all_ones)

            # ---- classifier head on the CLS token (token 0) ----
            with (
                tc.tile_pool(name="ps_cls", bufs=2, space="PSUM") as ps_cls,
                tc.tile_pool(name="cls", bufs=1) as clsp,
            ):
                # fp32r matmuls need even free sizes: run the CLS stats over
                # token columns 0:2 and use column 0.
                csum = ps_cls.tile([1, 2], F32, tag="cstat")
                cssq = ps_cls.tile([1, 2], F32, tag="cstat")
                sqc = []
                for kt in range(KT):
                    sq = clsp.tile([128, 2], F32R, tag=f"csq{kt}")
                    nc.scalar.square(sq[:], ht[kt][:, 0:2])
                    sqc.append(sq)
                for kt in range(KT):
                    nc.tensor.matmul(csum[:], ones[:], ht[kt][:, 0:2],
                                     start=(kt == 0), stop=(kt == KT - 1))
                for kt in range(KT):
                    nc.tensor.matmul(cssq[:], ones[:], sqc[kt][:],
                                     start=(kt == 0), stop=(kt == KT - 1))
                m = clsp.tile([1, 2], F32, tag="cm")
                nc.vector.tensor_scalar_mul(m[:], csum[:], 1.0 / D)
                msq = clsp.tile([1, 2], F32, tag="cmsq")
                nc.vector.tensor_mul(msq[:], m[:], m[:])
                var = clsp.tile([1, 2], F32, tag="cvar")
                nc.vector.scalar_tensor_tensor(var[:], cssq[:], 1.0 / D, msq[:],
                                               op0=OP.mult, op1=OP.subtract)
                vare = clsp.tile([1, 2], F32, tag="cvare")
                nc.vector.tensor_scalar_add(vare[:], var[:], float(EPS))
                rec = clsp.tile([1, 2], F32, tag="crec")
                nc.vector.reciprocal(rec[:], vare[:])
                istd = clsp.tile([1, 2], F32R, tag="cistd")
                nc.scalar.sqrt(istd[:], rec[:])
                mistd = clsp.tile([1, 2], F32R, tag="cmistd")
                nc.vector.tensor_mul(mistd[:], m[:], istd[:])
                istd_b = ps_cls.tile([128, 2], F32, tag="cbc")
                mistd_b = ps_cls.tile([128, 2], F32, tag="cbc")
                nc.tensor.matmul(istd_b[:], onesr[:], istd[:], start=True, stop=True)
                nc.tensor.matmul(mistd_b[:], onesr[:], mistd[:], start=True, stop=True)
                istd_s = clsp.tile([128, 1], F32, tag="cistds")
                nc.vector.tensor_copy(istd_s[:], istd_b[:, 0:1])
                mistd_s = clsp.tile([128, 1], F32, tag="cmistds")
                nc.vector.tensor_copy(mistd_s[:], mistd_b[:, 0:1])
                pout = ps_cls.tile([1, C], F32, tag="cout", bufs=1)
                for kt in range(KT):
                    cls0 = clsp.tile([128, 1], F32R, tag=f"cls0{kt}")
                    nc.vector.tensor_scalar(cls0[:], ht[kt][:, 0:1], istd_s[:],
                                            mistd_s[:], op0=OP.mult,
                                            op1=OP.subtract)
                    wc = clsp.tile([128, C], F32R, tag=f"wc{kt}")
                    nc.sync.dma_start(wc[:], WC[kt * 128:(kt + 1) * 128, :].bitcast(F32R))
                    nc.tensor.matmul(pout[:], cls0[:], wc[:],
                                     start=(kt == 0), stop=(kt == KT - 1))
                bc_sb = clsp.tile([1, C], F32, tag="cbias")
                nc.sync.dma_start(bc_sb[:], BC[:])
                orow = clsp.tile([1, C], F32, tag="orow")
                nc.vector.tensor_add(orow[:], pout[:], bc_sb[:])
                nc.sync.dma_start(OUT[:], orow[:])

    nc.compile()
    return nc


# --------------------------------------------------------------------------
# host side
# --------------------------------------------------------------------------

def _pos_enc():
    pos = np.arange(B, dtype=np.float32)[:, None]
    div = np.exp(np.arange(0, D, 2, dtype=np.float32) * (-np.log(10000.0) / D))
    pe = np.zeros((B, D), np.float32)
    pe[:, 0::2] = np.sin(pos * div)
    pe[:, 1::2] = np.cos(pos * div)
    return pe


def _prep_shared(inputs):
    """Fold LN gains/biases into adjacent weights; restage for the device."""
    f32 = lambda a: np.ascontiguousarray(np.asarray(a), dtype=np.float32)
    Wq, bq = f32(inputs["Wq"]), f32(inputs["bq"])
    Wk, bk = f32(inputs["Wk"]), f32(inputs["bk"])
    Wv, bv = f32(inputs["Wv"]), f32(inputs["bv"])
    Wo, bo = f32(inputs["Wo"]), f32(inputs["bo"])
    W1, bf1 = f32(inputs["W1"]), f32(inputs["bf1"])
    W2, bf2 = f32(inputs["W2"]), f32(inputs["bf2"])
    g1, b1 = f32(inputs["ln1_g"]), f32(inputs["ln1_b"])
    g2, b2 = f32(inputs["ln2_g"]), f32(inputs["ln2_b"])
    scale = np.float32(1.0 / np.sqrt(HD))

    wq = np.empty((L, D, D), np.float32)
    wk = np.empty((L, D, D), np.float32)
    wv = np.empty((L, D, D), np.float32)
    w1r = np.empty((L, FT, 128, D), np.float32)
    bqc = np.empty((L, 128, KT), np.float32)
    bkc = np.empty((L, 128, KT), np.float32)
    boc = np.empty((L, 128, KT), np.float32)
    bf1c = np.empty((L, 128, FT), np.float32)
    bf2c = np.empty((L, 128, KT), np.float32)
    col = lambda v, n: v.reshape(n, 128).T

    for l in range(L):
        wq[l] = (g1[l][:, None] * Wq[l]) * scale
        wk[l] = g1[l][:, None] * Wk[l]
        wv[l] = g1[l][:, None] * Wv[l]
        bql = (bq[l] + b1[l] @ Wq[l]) * scale
        bkl = bk[l] + b1[l] @ Wk[l]
        bvl = bv[l] + b1[l] @ Wv[l]
        bol = bo[l] + bvl @ Wo[l]
        w1l = g2[l][:, None] * W1[l]
        bf1l = bf1[l] + b2[l] @ W1[l]
        for i in range(FT):
            for kt in range(KT):
                w1r[l, i, :, kt * 128:(kt + 1) * 128] = \
                    w1l[kt * 128:(kt + 1) * 128, i * 128:(i + 1) * 128]
        bqc[l], bkc[l], boc[l] = col(bql, KT), col(bkl, KT), col(bol, KT)
        bf1c[l], bf2c[l] = col(bf1l, FT), col(bf2[l], KT)

    lncg, lncb = f32(inputs["lnc_g"]), f32(inputs["lnc_b"])
    Wc, bc = f32(inputs["Wc"]), f32(inputs["bc"])
    wc = lncg[:, None] * Wc
    bcf = (bc + lncb @ Wc).reshape(1, C)

    return {
        "wq": wq, "wk": wk, "wv": wv, "wo": Wo,
        "w1r": w1r, "w2": W2,
        "bq": bqc, "bk": bkc, "bo": boc, "bf1": bf1c, "bf2": bf2c,
        "wc": np.ascontiguousarray(wc), "bc": np.ascontiguousarray(bcf),
        "ones_col": np.ones((128, 1), np.float32),
        "ones_row": np.ones((1, 128), np.float32),
    }


def kernel(**inputs):
    x = np.asarray(inputs["x"])
    mask = np.asarray(inputs["mask"])
    emb = np.ascontiguousarray(np.asarray(inputs["emb"]), dtype=np.float32)
    mask_all_ones = bool(np.all(mask != 0))

    shared = _prep_shared(inputs)
    pe = _pos_enc()

    key = ("prog", mask_all_ones)
    if key not in _CACHE:
        _CACHE[key] = _build(mask_all_ones)
    nc = _CACHE[key]

    in_maps = []
    for b in range(B):
        m = dict(shared)
        h0 = emb[x[b]] + pe[b][None, :]                       # [S, D] fp32
        m["h0t"] = np.ascontiguousarray(h0.T)                 # [D, S]
        kb = np.where(mask[b] != 0, 0.0, -1e9).astype(np.float32)
        m["kbcol"] = np.ascontiguousarray(kb.reshape(ST, 128).T)
        m["mask01"] = (mask[b] != 0).astype(np.float32).reshape(1, S)
        in_maps.append(m)

    trace = bool(os.environ.get("BERT_TRACE"))
    if trace:
        _install_trace_shim()
    res = run_bass_kernel_spmd(nc, in_maps, core_ids=list(range(N_CORES)),
                               trace=trace)
    if trace:
        print(f"HW exec time: {res.exec_time_ns} ns")

    out = np.concatenate([res.results[b]["out"] for b in range(B)], axis=0)
    attns = np.stack([res.results[b]["attn"] for b in range(B)], axis=1)
    return out, attns


def _install_trace_shim():
    """The agent image's antenv lacks axon_hooks; register the NTFF profile
    hook directly from the boot helpers so trace=True works."""
    import types
    if "antenv.axon_hooks" in sys.modules:
        return
    try:
        from trn_agent_boot.trn_boot import _ntff_profile_via_ctypes
        hook = _ntff_profile_via_ctypes("/opt/axon/libaxon_pjrt.so")
    except Exception:
        hook = None
    mod = types.ModuleType("antenv.axon_hooks")
    mod.get_axon_ntff_profile_hook = lambda: hook
    sys.modules["antenv.axon_hooks"] = mod
